# revision 1
# baseline (speedup 1.0000x reference)
"""Trainium2 Bass kernel: additive-attention MultiHeadAttention (B=32,Q=8,K=2048,D=256,H=8).

Self-contained: hardcodes shapes and the batch-parallel sharding (4 batches per core
across 8 NeuronCores).  kernel(**inputs) takes full unsharded inputs and returns the
full [32, 256] output.

Math per core (b = 4 local batches):
  qp[b,q,j]   = queries @ Wq.T
  kT[j,k]     = (keys @ Wk.T).T          (PE, output transposed: j on partitions)
  vp[k,j]     = values @ Wv.T            (PE, natural: k on partitions)
  feat        = tanh(kT + qp)            (ACT: bias = per-partition qp column)
  scoresT[k,(q,h)] = feat.T @ S          (PE: feat is the *stationary* operand,
                                          S[(h,d),h'] = wv[d]*delta(h,h') folds the
                                          wv reduction; output has k on partitions)
  softmax over q: free-dim reduce + reciprocal + broadcast multiply (DVE)
  aoT[j',q]   = vp.T @ en                (PE, col-tiled; j' on partitions)
  out2T       = WoT.T @ aoT ; y = fc(out2) + fcb
"""

import numpy as np

import concourse.bacc as bacc
import concourse.bass as bass
import concourse.mybir as mybir
import concourse.tile as tile
from concourse.bass_utils import run_bass_kernel_spmd
from concourse.masks import make_identity

# Problem shapes (full problem; hardcoded per the harness contract)
B, Q, KL, D = 32, 8, 2048, 256
H, DH = 8, 32
NCORES = 8
NB = B // NCORES  # 4 batches per core
KC = KL // 128    # 16 kpos chunks
F32 = mybir.dt.float32
BF16 = mybir.dt.bfloat16
Tanh = mybir.ActivationFunctionType.Tanh
Exp = mybir.ActivationFunctionType.Exp


def _emit(tc):
    nc = tc.nc

    # ------------------------------------------------------------------ I/O
    queries = nc.dram_tensor("queries", [NB, Q, D], F32, kind="ExternalInput").ap()
    keys = nc.dram_tensor("keys", [NB, KL, D], F32, kind="ExternalInput").ap()
    values = nc.dram_tensor("values", [NB, KL, D], F32, kind="ExternalInput").ap()
    Wq = nc.dram_tensor("Wq", [D, D], F32, kind="ExternalInput").ap()
    Wk = nc.dram_tensor("Wk", [D, D], F32, kind="ExternalInput").ap()
    Wv = nc.dram_tensor("Wv", [D, D], F32, kind="ExternalInput").ap()
    Wo = nc.dram_tensor("Wo", [D, D], F32, kind="ExternalInput").ap()
    wv_score = nc.dram_tensor("wv_score", [DH], F32, kind="ExternalInput").ap()
    fcW = nc.dram_tensor("fcW", [D, Q * D], F32, kind="ExternalInput").ap()
    fcb = nc.dram_tensor("fcb", [D], F32, kind="ExternalInput").ap()
    out = nc.dram_tensor("out", [NB, D], F32, kind="ExternalOutput").ap()

    # ------------------------------------------------------------------ pools
    dram = tc.alloc_tile_pool(name="dram", bufs=1, space="DRAM")
    consts = tc.alloc_tile_pool(name="consts", bufs=1)
    psA = tc.alloc_tile_pool(name="psA", bufs=1, space="PSUM")
    vp_pool = tc.alloc_tile_pool(name="vp_ps", bufs=2, space="PSUM")
    krep_pool = tc.alloc_tile_pool(name="krep_ps", bufs=1, space="PSUM")
    sc_pool = tc.alloc_tile_pool(name="sc_ps", bufs=2, space="PSUM")
    ao_pool = tc.alloc_tile_pool(name="ao_ps", bufs=1, space="PSUM")
    krepsb_pool = tc.alloc_tile_pool(name="krep_sb", bufs=4)
    feat_pool = tc.alloc_tile_pool(name="feat", bufs=4)
    soft_pool = tc.alloc_tile_pool(name="soft", bufs=2)
    pools = [
        soft_pool, feat_pool, krepsb_pool, ao_pool, sc_pool,
        krep_pool, vp_pool, psA, consts, dram,
    ]

    # ---------------------------------------------- constants & table preload
    id32b = consts.tile([32, 32], BF16, tag="id32b", name="id32b")
    id32f = consts.tile([32, 32], F32, tag="id32f", name="id32f")
    make_identity(nc, id32b[:])
    make_identity(nc, id32f[:])
    # dummy activation to pull the exp/tanh table load off the critical path
    dummy = consts.tile([1, 2], F32, tag="dummy", name="dummy")
    nc.vector.memset(dummy[:], 0.0)
    nc.scalar.activation(out=dummy[:], in_=dummy[:], func=Tanh)

    # S[(hh,d), h'] = wv_score[d] * delta(hh, h')   (hh = head-within-half)
    S_f32 = consts.tile([128, 4], F32, tag="S_f32", name="S_f32")
    S = consts.tile([128, 4], BF16, tag="S", name="S")
    nc.vector.memset(S_f32[:], 0.0)
    wv_col = wv_score.rearrange("(d one) -> d one", one=1)
    for hh in range(4):
        nc.sync.dma_start(out=S_f32[hh * 32 : (hh + 1) * 32, hh : hh + 1], in_=wv_col)
    nc.vector.tensor_copy(out=S[:], in_=S_f32[:])

    fcb_sb = consts.tile([NB, D], F32, tag="fcb_sb", name="fcb_sb")
    fcb_b = bass.AP(tensor=fcb.tensor, offset=fcb.offset, ap=[[0, NB], [1, D]])
    nc.sync.dma_start(out=fcb_sb[:], in_=fcb_b)

    # ------------------------------------------------------------ query path
    # (critical: produces the per-partition tanh bias columns)
    wq_bf = dram.tile([D, D], BF16, tag="wq_bf", name="wq_bf")
    wk_bf = dram.tile([D, D], BF16, tag="wk_bf", name="wk_bf")
    keys_bf = dram.tile([NB, KL, D], BF16)
    nc.gpsimd.dma_start(out=wq_bf[:], in_=Wq)
    nc.gpsimd.dma_start(out=wk_bf[:], in_=Wk)
    nc.gpsimd.dma_start(out=keys_bf[0], in_=keys[0])

    def wtrans(name, src):
        ts = [consts.tile([128, D], BF16, tag=f"{name}{ch}", name=f"{name}{ch}") for ch in range(2)]
        for ch in range(2):
            nc.sync.dma_start(
                out=ts[ch][:], in_=src[:, ch * 128 : (ch + 1) * 128], transpose=True
            )
        return ts

    WqT = wtrans("WqT", wq_bf)   # WqT[ch][c_lo, j] = Wq[j, ch*128+c_lo]

    q_nat = consts.tile([NB * Q, D], BF16, tag="q_nat", name="q_nat")
    nc.gpsimd.dma_start(out=q_nat[:], in_=queries.rearrange("b q d -> (b q) d"))
    qT = [consts.tile([128, NB * Q], BF16, tag=f"qT{ch}", name=f"qT{ch}") for ch in range(2)]
    for ch in range(2):
        qT_ps = psA.tile([128, NB * Q], BF16, tag="psA", name="qT_ps")
        nc.tensor.transpose(
            out=qT_ps[:], in_=q_nat[:, ch * 128 : (ch + 1) * 128], identity=id32b[:]
        )
        nc.vector.tensor_copy(out=qT[ch][:], in_=qT_ps[:])
    # q_projT[hg][j_lo, (b,q)] directly: lhsT = WqT j-half (stationary), rhs = queriesT
    qpT = [consts.tile([128, NB * Q], F32, tag=f"qpT{hg}", name=f"qpT{hg}") for hg in range(2)]
    for hg in range(2):
        qpT_ps = psA.tile([128, NB * Q], F32, tag="psA", name="qpT_ps")
        for ch in range(2):
            nc.tensor.matmul(
                out=qpT_ps[:],
                lhsT=WqT[ch][:, hg * 128 : (hg + 1) * 128],
                rhs=qT[ch][:],
                start=(ch == 0),
                stop=(ch == 1),
            )
        nc.vector.tensor_copy(out=qpT[hg][:], in_=qpT_ps[:])

    # ------------------------------------------------- Wk/Wv weights
    wv_bf = dram.tile([D, D], BF16, tag="wv_bf", name="wv_bf")
    nc.gpsimd.dma_start(out=wv_bf[:], in_=Wv)
    WkT = wtrans("WkT", wk_bf)
    WvT = wtrans("WvT", wv_bf)

    values_bf = dram.tile([NB, KL, D], BF16)
    keysT = [
        [consts.tile([128, KL], BF16, tag=f"keysT{b}_{ch}", name=f"keysT{b}_{ch}") for ch in range(2)]
        for b in range(NB)
    ]
    valuesT = [
        [consts.tile([128, KL], BF16, tag=f"valuesT{b}_{ch}", name=f"valuesT{b}_{ch}") for ch in range(2)]
        for b in range(NB)
    ]
    v_sb = [consts.tile([128, NB * D], BF16, tag=f"v_sb{kc}", name=f"v_sb{kc}") for kc in range(KC)]
    aoT = [consts.tile([128, NB * Q], BF16, tag=f"aoT{hg}", name=f"aoT{hg}") for hg in range(2)]

    def emit_keys_chain(b, hold=None):
        if b > 0:
            cast = nc.gpsimd.dma_start(out=keys_bf[b], in_=keys[b])
            if hold is not None:
                tile.add_dep_helper(cast.ins, hold.ins, reason="dma order")
            tr = None
            for ch in range(2):
                tr = nc.sync.dma_start(
                    out=keysT[b][ch][:],
                    in_=keys_bf[b, :, ch * 128 : (ch + 1) * 128],
                    transpose=True,
                )
            return tr
        tr = None
        for ch in range(2):
            tr = nc.sync.dma_start(
                out=keysT[b][ch][:],
                in_=keys_bf[b, :, ch * 128 : (ch + 1) * 128],
                transpose=True,
            )
        return tr

    def emit_kproj(b, hg):
        # k-projT for this (batch, head-half): [128=(hh,dh), KL] fp32
        krep_sb = krepsb_pool.tile([128, KL], F32, name="krep_sb")
        for half in range(2):
            krep_ps = krep_pool.tile([128, KL // 2], F32, tag="krep", name="krep_ps")
            for nch in range(2):
                nco = half * 2 + nch
                for ch in range(2):
                    nc.tensor.matmul(
                        out=krep_ps[:, nch * 512 : (nch + 1) * 512],
                        lhsT=WkT[ch][:, hg * 128 : (hg + 1) * 128],
                        rhs=keysT[b][ch][:, nco * 512 : (nco + 1) * 512],
                        start=(ch == 0),
                        stop=(ch == 1),
                    )
            nc.vector.tensor_copy(
                out=krep_sb[:, half * (KL // 2) : (half + 1) * (KL // 2)],
                in_=krep_ps[:],
            )
        return krep_sb

    def emit_values_chain(b, hold=None):
        cast = nc.gpsimd.dma_start(out=values_bf[b], in_=values[b])
        if hold is not None:
            tile.add_dep_helper(cast.ins, hold.ins, reason="dma order")
        tr = None
        for ch in range(2):
            tr = nc.sync.dma_start(
                out=valuesT[b][ch][:],
                in_=values_bf[b, :, ch * 128 : (ch + 1) * 128],
                transpose=True,
            )
        return tr

    def emit_vproj(b):
        for kc in range(KC):
            vp_ps = vp_pool.tile([128, D], F32)
            for ch in range(2):
                nc.tensor.matmul(
                    out=vp_ps[:],
                    lhsT=valuesT[b][ch][:, kc * 128 : (kc + 1) * 128],
                    rhs=WvT[ch][:],
                    start=(ch == 0),
                    stop=(ch == 1),
                )
            nc.vector.tensor_copy(out=v_sb[kc][:, b * D : (b + 1) * D], in_=vp_ps[:])

    # ------------------------------------------------------------- main loop
    def emit_main(b, hg, krep_sb):

        # scoresT accumulate into one bank: free layout (kc, q, hh)
        sc_ps = sc_pool.tile([128, 512], F32)
        sc_r = sc_ps[:].rearrange("p (kc q h) -> p kc q h", kc=KC, q=Q, h=4)
        for q in range(Q):
            feat = feat_pool.tile([128, KL], BF16)
            nc.scalar.activation(
                out=feat[:],
                in_=krep_sb[:],
                func=Tanh,
                bias=qpT[hg][:, b * Q + q : b * Q + q + 1],
            )
            for kc in range(KC):
                nc.tensor.matmul(
                    out=sc_r[:, kc, q, :],
                    lhsT=feat[:, kc * 128 : (kc + 1) * 128],
                    rhs=S[:],
                    start=True,
                    stop=True,
                )

        if hg == 0:
            emit_vproj(b)

        # softmax over q (free-dim): exp -> Z -> 1/Z -> en = exp * invZ
        exp_sb = soft_pool.tile([128, 512], F32, tag="exp_sb", name="exp_sb")
        nc.scalar.activation(out=exp_sb[:], in_=sc_ps[:], func=Exp)
        Zt = soft_pool.tile([128, 64], F32, tag="Zt", name="Zt")
        exp_khq = exp_sb[:].rearrange("p (kc q h) -> p kc h q", kc=KC, q=Q, h=4)
        nc.vector.tensor_reduce(
            out=Zt[:], in_=exp_khq, axis=mybir.AxisListType.X, op=mybir.AluOpType.add
        )
        invZ = soft_pool.tile([128, 64], F32, tag="invZ", name="invZ")
        nc.vector.reciprocal(out=invZ[:], in_=Zt[:])
        en = soft_pool.tile([128, 512], BF16, tag="en", name="en")
        in0 = exp_sb[:].rearrange("p (kc q h) -> p kc q h", kc=KC, q=Q, h=4)
        iz = invZ[:].rearrange("p (kc h) -> p kc h", kc=KC, h=4)
        in1 = bass.AP(
            tensor=iz.tensor,
            offset=iz.offset,
            ap=[list(iz.ap[0]), list(iz.ap[1]), [0, Q], list(iz.ap[2])],
        )
        en_r = en[:].rearrange("p (kc q h) -> p kc q h", kc=KC, q=Q, h=4)
        nc.vector.tensor_tensor(out=en_r, in0=in0, in1=in1, op=mybir.AluOpType.mult)

        # attn @ v, transposed out: aoT_ps[hh*32+dh, q] for the 4 heads of hg
        ao_ps = ao_pool.tile([128, Q], F32)
        prev_group_last = None
        for hh in range(4):
            j0 = b * D + (hg * 4 + hh) * DH
            for kc in range(KC):
                mm = nc.tensor.matmul(
                    out=ao_ps[hh * 32 : (hh + 1) * 32, :],
                    lhsT=v_sb[kc][:, j0 : j0 + DH],
                    rhs=en_r[:, kc, :, hh],
                    start=(kc == 0),
                    stop=(kc == KC - 1),
                    tile_position=(0, hh * 32),
                    skip_group_check=True,
                )
                # keep accumulation groups sequential on PE
                if prev_group_last is not None:
                    tile.add_dep_helper(
                        mm.ins,
                        prev_group_last,
                        sync=False,
                        reason="ao accumulation group order",
                    )
                prev_group_last = mm.ins
        nc.vector.tensor_copy(out=aoT[hg][:, b * Q : (b + 1) * Q], in_=ao_ps[:])


    # software-pipelined driver: batch b+1's projections emitted between
    # batch b's two tanh/score rounds
    keys_tr = emit_keys_chain(0)
    kreps = [emit_kproj(0, 0), emit_kproj(0, 1)]
    last_tr = emit_values_chain(0, hold=keys_tr)
    for b in range(NB):
        if b + 1 < NB:
            next_keys_tr = emit_keys_chain(b + 1, hold=last_tr)
        emit_main(b, 0, kreps[0])
        if b + 1 < NB:
            next_kreps = [emit_kproj(b + 1, 0), emit_kproj(b + 1, 1)]
            last_tr = emit_values_chain(b + 1, hold=next_keys_tr)
        emit_main(b, 1, kreps[1])
        if b + 1 < NB:
            kreps = next_kreps

    # -------------------------------------------------- tail weights (late)
    wo_bf = dram.tile([D, D], BF16, tag="wo_bf", name="wo_bf")
    fcw_bf = dram.tile([D, Q * D], BF16, tag="fcw_bf", name="fcw_bf")
    wo_cast = nc.gpsimd.dma_start(out=wo_bf[:], in_=Wo)
    tile.add_dep_helper(wo_cast.ins, last_tr.ins, reason="dma order")
    fcw_cast = nc.gpsimd.dma_start(out=fcw_bf[:], in_=fcW)
    tile.add_dep_helper(fcw_cast.ins, wo_cast.ins, reason="dma order")
    WoT = wtrans("WoT", wo_bf)   # WoT[ch][jp_lo, jo] = Wo[jo, ch*128+jp_lo]
    fcwT = [consts.tile([128, D], BF16, tag=f"fcwT{t}", name=f"fcwT{t}") for t in range(16)]
    for t in range(16):
        nc.sync.dma_start(
            out=fcwT[t][:], in_=fcw_bf[:, t * 128 : (t + 1) * 128], transpose=True
        )

    # ------------------------------------------------------------------ tail
    # out2T[m][jo_lo, (b,q)] = (ao @ Wo.T) transposed
    o2T = [consts.tile([128, NB * Q], BF16, tag=f"o2T{m}", name=f"o2T{m}") for m in range(2)]
    for m in range(2):
        o2_ps = psA.tile([128, NB * Q], F32, tag="psA", name="o2_ps")
        for ch in range(2):
            nc.tensor.matmul(
                out=o2_ps[:],
                lhsT=WoT[ch][:, m * 128 : (m + 1) * 128],
                rhs=aoT[ch][:],
                start=(ch == 0),
                stop=(ch == 1),
            )
        nc.vector.tensor_copy(out=o2T[m][:], in_=o2_ps[:])

    # fc: y[b, f] = sum_{q,jo} out2[b,q,jo] * fcW[f, q*256+jo]
    y_ps = psA.tile([NB, D], F32, tag="psA", name="y_ps")
    for t in range(16):
        qq, m = t // 2, t % 2
        lhsT = o2T[m][:].rearrange("p (b q) -> p q b", b=NB, q=Q)[:, qq, :]
        nc.tensor.matmul(
            out=y_ps[:], lhsT=lhsT, rhs=fcwT[t][:], start=(t == 0), stop=(t == 15)
        )
    y_sb = consts.tile([NB, D], F32, tag="y_sb", name="y_sb")
    nc.vector.tensor_tensor(
        out=y_sb[:], in0=y_ps[:], in1=fcb_sb[:], op=mybir.AluOpType.add
    )
    nc.sync.dma_start(out=out, in_=y_sb[:])

    for p in pools:
        p.release()


_NC_CACHE = None


def _get_nc():
    global _NC_CACHE
    if _NC_CACHE is None:
        nc = bacc.Bacc(
            "TRN2", target_bir_lowering=False, debug=False, num_devices=NCORES
        )
        with tile.TileContext(nc) as tc:
            _emit(tc)
        nc.compile()
        _NC_CACHE = nc
    return _NC_CACHE


def _in_maps(inputs):
    f32 = lambda x: np.ascontiguousarray(np.asarray(x), dtype=np.float32)
    queries = f32(inputs["queries"])
    keys = f32(inputs["keys"])
    values = f32(inputs["values"])
    shared = {
        "Wq": f32(inputs["Wq"]),
        "Wk": f32(inputs["Wk"]),
        "Wv": f32(inputs["Wv"]),
        "Wo": f32(inputs["Wo"]),
        "wv_score": f32(inputs["wv_score"]),
        "fcW": f32(inputs["fcW"]),
        "fcb": f32(inputs["fcb"]),
    }
    maps = []
    for c in range(NCORES):
        sl = slice(c * NB, (c + 1) * NB)
        maps.append(
            {
                "queries": np.ascontiguousarray(queries[sl]),
                "keys": np.ascontiguousarray(keys[sl]),
                "values": np.ascontiguousarray(values[sl]),
                **shared,
            }
        )
    return maps


def run(inputs, trace=False):
    nc = _get_nc()
    res = run_bass_kernel_spmd(
        nc, _in_maps(inputs), core_ids=list(range(NCORES)), trace=trace
    )
    outp = np.concatenate([res.results[c]["out"] for c in range(NCORES)], axis=0)
    return outp, res.exec_time_ns


def run_sim(inputs):
    """Simulate core 0 only (CoreSim); returns the [NB, D] slice."""
    import concourse.bass_interp as bass_interp

    nc = _get_nc()
    sim = bass_interp.CoreSim(nc)
    for k, v in _in_maps(inputs)[0].items():
        sim.tensor(k)[:] = v
    sim.simulate()
    return np.array(sim.tensor("out"))


def kernel(**inputs):
    return run(inputs, trace=False)[0]



# revision 2
# speedup vs baseline: 1.0651x; 1.0651x over previous
"""Trainium2 Bass kernel: additive-attention MHA (B=32,Q=8,K=2048,D=256,H=8).

v2: ACT/DVE split feature computation via tanh addition identity
    tanh(q+k) = (a+u)/(1+a*u),  a=tanh(qp) per-partition scalar, u=tanh(kp).
    Values path: attn@values first (values natural, no transpose/projection),
    Wv and Wo applied to the tiny (64, 256) result afterwards.

Sharding: batch-parallel, 4 batches per core across 8 cores.
"""

import numpy as np

import concourse.bacc as bacc
import concourse.bass as bass
import concourse.mybir as mybir
import concourse.tile as tile
from concourse.bass_utils import run_bass_kernel_spmd
from concourse.masks import make_identity

B, Q, KL, D = 32, 8, 2048, 256
H, DH = 8, 32
NCORES = 8
NB = B // NCORES  # 4
KC = KL // 128    # 16
F32 = mybir.dt.float32
BF16 = mybir.dt.bfloat16
Tanh = mybir.ActivationFunctionType.Tanh
Exp = mybir.ActivationFunctionType.Exp
MULT = mybir.AluOpType.mult
ADD = mybir.AluOpType.add

# Column split of the (128, 2048) feature tile per (b, hg, q):
#   [0, X_END)       ACT direct tanh(kp + qp)
#   [X_END, P_SPLIT) DVE Mobius (num/den on DVE)
#   [P_SPLIT, 2048)  Mobius with num/den on Pool, recip/mult on DVE
X_END = 1520
P_SPLIT = 1856  # cols >= P_SPLIT: feat-mult on Pool
W = KL - X_END  # Mobius width


def _emit(tc):
    nc = tc.nc

    # ------------------------------------------------------------------ I/O
    queries = nc.dram_tensor("queries", [NB, Q, D], F32, kind="ExternalInput").ap()
    keys = nc.dram_tensor("keys", [NB, KL, D], F32, kind="ExternalInput").ap()
    values = nc.dram_tensor("values", [NB, KL, D], F32, kind="ExternalInput").ap()
    Wq = nc.dram_tensor("Wq", [D, D], F32, kind="ExternalInput").ap()
    Wk = nc.dram_tensor("Wk", [D, D], F32, kind="ExternalInput").ap()
    Wv = nc.dram_tensor("Wv", [D, D], F32, kind="ExternalInput").ap()
    Wo = nc.dram_tensor("Wo", [D, D], F32, kind="ExternalInput").ap()
    wv_score = nc.dram_tensor("wv_score", [DH], F32, kind="ExternalInput").ap()
    fcW = nc.dram_tensor("fcW", [D, Q * D], F32, kind="ExternalInput").ap()
    fcb = nc.dram_tensor("fcb", [D], F32, kind="ExternalInput").ap()
    out = nc.dram_tensor("out", [NB, D], F32, kind="ExternalOutput").ap()

    # ------------------------------------------------------------------ pools
    dram = tc.alloc_tile_pool(name="dram", bufs=1, space="DRAM")
    consts = tc.alloc_tile_pool(name="consts", bufs=1)
    psA = tc.alloc_tile_pool(name="psA", bufs=1, space="PSUM")
    kp_pool = tc.alloc_tile_pool(name="kp_ps", bufs=2, space="PSUM")
    sc_pool = tc.alloc_tile_pool(name="sc_ps", bufs=2, space="PSUM")
    small_ps = tc.alloc_tile_pool(name="small_ps", bufs=1, space="PSUM")
    kpsb_pool = tc.alloc_tile_pool(name="kp_sb", bufs=4)
    u_pool = tc.alloc_tile_pool(name="u_sb", bufs=4)
    mob_pool = tc.alloc_tile_pool(name="mob", bufs=4)
    feat_pool = tc.alloc_tile_pool(name="feat", bufs=8)
    soft_pool = tc.alloc_tile_pool(name="soft", bufs=3)
    en_pool = tc.alloc_tile_pool(name="en", bufs=2)
    pools = [
        en_pool, soft_pool, feat_pool, mob_pool, u_pool, kpsb_pool,
        small_ps, sc_pool, kp_pool, psA, consts, dram,
    ]

    # keys0 cast first: longest pole of the startup critical path
    keys_bf = dram.tile([NB, KL, D], BF16)
    nc.gpsimd.dma_start(
        out=keys_bf[0].rearrange("k c -> (k c)"),
        in_=keys[0].rearrange("k c -> (k c)"),
    )

    # ---------------------------------------------- constants & table preload
    id32b = consts.tile([32, 32], BF16, tag="id32b", name="id32b")
    make_identity(nc, id32b[:])
    dummy = consts.tile([1, 2], F32, tag="dummy", name="dummy")
    nc.vector.memset(dummy[:], 0.0)
    nc.scalar.activation(out=dummy[:], in_=dummy[:], func=Tanh)

    # ------------------------------------------------------------ query path
    id128f = consts.tile([128, 128], F32, tag="id128f", name="id128f")
    make_identity(nc, id128f[:])

    def wtrans(name, W_ap, dep=None, copy_eng=None):
        """WT[ch][c_lo, j] = W[j, ch*128 + c_lo] via natural f32 load + PE."""
        copy_eng = copy_eng or nc.scalar
        nat = []
        for r in range(2):
            t = consts.tile([128, D], F32, tag=f"{name}nat{r}", name=f"{name}nat{r}")
            dma = nc.sync.dma_start(out=t[:], in_=W_ap[r * 128 : (r + 1) * 128, :])
            if dep is not None:
                tile.add_dep_helper(dma.ins, dep.ins, reason="dma order")
            nat.append(t)
        ts = []
        for ch in range(2):
            ps = psA.tile([128, D], F32, tag="psA", name=f"{name}ps{ch}")
            for r in range(2):
                nc.tensor.transpose(
                    out=ps[:, r * 128 : (r + 1) * 128],
                    in_=nat[r][:, ch * 128 : (ch + 1) * 128],
                    identity=id128f[:],
                )
            t = consts.tile([128, D], BF16, tag=f"{name}{ch}", name=f"{name}{ch}")
            if copy_eng is nc.scalar:
                nc.scalar.copy(out=t[:], in_=ps[:])
            else:
                copy_eng.tensor_copy(out=t[:], in_=ps[:])
            ts.append(t)
        return ts

    WqT = wtrans("WqT", Wq)

    q_nat = consts.tile([NB * Q, D], BF16, tag="q_nat", name="q_nat")
    nc.gpsimd.dma_start(out=q_nat[:], in_=queries.rearrange("b q d -> (b q) d"))
    qT = [
        consts.tile([128, NB * Q], BF16, tag=f"qT{ch}", name=f"qT{ch}")
        for ch in range(2)
    ]
    for ch in range(2):
        qT_ps = psA.tile([128, NB * Q], BF16, tag="psA", name="qT_ps")
        nc.tensor.transpose(
            out=qT_ps[:], in_=q_nat[:, ch * 128 : (ch + 1) * 128], identity=id32b[:]
        )
        nc.vector.tensor_copy(out=qT[ch][:], in_=qT_ps[:])
    # qpT[hg][j_lo, (b,q)] f32 (ACT bias), and a = tanh(qpT) f32 (Mobius scalar)
    qpT = [
        consts.tile([128, NB * Q], F32, tag=f"qpT{hg}", name=f"qpT{hg}")
        for hg in range(2)
    ]
    a_t = [
        consts.tile([128, NB * Q], F32, tag=f"a{hg}", name=f"a{hg}")
        for hg in range(2)
    ]
    for hg in range(2):
        qpT_ps = psA.tile([128, NB * Q], F32, tag="psA", name="qpT_ps")
        for ch in range(2):
            nc.tensor.matmul(
                out=qpT_ps[:],
                lhsT=WqT[ch][:, hg * 128 : (hg + 1) * 128],
                rhs=qT[ch][:],
                start=(ch == 0),
                stop=(ch == 1),
            )
        nc.vector.tensor_copy(out=qpT[hg][:], in_=qpT_ps[:])
        nc.scalar.activation(out=a_t[hg][:], in_=qpT[hg][:], func=Tanh)

    # ------------------------------------------------- keys / values loading
    WkT = wtrans("WkT", Wk)

    # S[(hh,d), h'] = wv_score[d] * delta(hh, h')
    S_f32 = consts.tile([128, 4], F32, tag="S_f32", name="S_f32")
    S = consts.tile([128, 4], BF16, tag="S", name="S")
    nc.vector.memset(S_f32[:], 0.0)
    wv_col = wv_score.rearrange("(d one) -> d one", one=1)
    for hh in range(4):
        nc.scalar.dma_start(
            out=S_f32[hh * 32 : (hh + 1) * 32, hh : hh + 1], in_=wv_col
        )
    nc.vector.tensor_copy(out=S[:], in_=S_f32[:])

    fcb_sb = consts.tile([NB, D], F32, tag="fcb_sb", name="fcb_sb")
    fcb_b = bass.AP(tensor=fcb.tensor, offset=fcb.offset, ap=[[0, NB], [1, D]])
    nc.scalar.dma_start(out=fcb_sb[:], in_=fcb_b)


    keysT = [
        [
            consts.tile([128, KL], BF16, tag=f"keysT{b}_{ch}", name=f"keysT{b}_{ch}")
            for ch in range(2)
        ]
        for b in range(NB)
    ]
    # values natural: v_all[p, (b, kc, c)] = values[b, kc*128+p, c]  (bf16)
    v_all = consts.tile([128, NB * KC * D], BF16, tag="v_all", name="v_all")
    values_bf = dram.tile([NB, KL, D], BF16)

    def emit_keys_cast(b, dep=None):
        dma = nc.gpsimd.dma_start(
            out=keys_bf[b].rearrange("k c -> (k c)"),
            in_=keys[b].rearrange("k c -> (k c)"),
        )
        if dep is not None:
            tile.add_dep_helper(dma.ins, dep.ins, reason="dma order")
        return dma

    def emit_keys_tr(b):
        tr = None
        for ch in range(2):
            tr = nc.sync.dma_start(
                out=keysT[b][ch][:],
                in_=keys_bf[b, :, ch * 128 : (ch + 1) * 128],
                transpose=True,
            )
        return tr

    def emit_values(b, dep=None):
        cast = nc.gpsimd.dma_start(
            out=values_bf[b].rearrange("k c -> (k c)"),
            in_=values[b].rearrange("k c -> (k c)"),
        )
        if dep is not None:
            tile.add_dep_helper(cast.ins, dep.ins, reason="dma order")
        # src[p, kc, c] = values_bf[b, kc*128 + p, c]
        vb = values_bf[b]
        src = bass.AP(
            tensor=vb.tensor,
            offset=vb.offset,
            ap=[[D, 128], [128 * D, KC], [1, D]],
        )
        dst = v_all[:, b * KC * D : (b + 1) * KC * D].rearrange(
            "p (kc c) -> p kc c", kc=KC
        )
        return nc.sync.dma_start(out=dst, in_=src)

    # ------------------------------------------------------------- main loop
    en_tiles = {}

    def emit_kp(b, hg):
        """k-projection + SBUF staging, half-KL granular (2 PSUM banks/half)."""
        kp_sb = kpsb_pool.tile([128, KL], BF16, name="kp_sb")
        for half in range(2):
            kp_ps = kp_pool.tile([128, KL // 2], F32, tag="kp", name="kp_ps")
            for nco in range(2):
                o = half * 2 + nco
                for ch in range(2):
                    nc.tensor.matmul(
                        out=kp_ps[:, nco * 512 : (nco + 1) * 512],
                        lhsT=WkT[ch][:, hg * 128 : (hg + 1) * 128],
                        rhs=keysT[b][ch][:, o * 512 : (o + 1) * 512],
                        start=(ch == 0),
                        stop=(ch == 1),
                    )
            nc.vector.tensor_copy(
                out=kp_sb[:, half * (KL // 2) : (half + 1) * (KL // 2)],
                in_=kp_ps[:],
            )
        return kp_sb

    def emit_exp(sc_ps):
        exp_sb = soft_pool.tile([128, KC * Q * 4], BF16, tag="exp", name="exp_sb")
        nc.scalar.activation(out=exp_sb[:], in_=sc_ps[:], func=Exp)
        return exp_sb

    def emit_u(kp_sb):
        u_sb = u_pool.tile([128, W], BF16, name="u_sb")
        nc.scalar.activation(out=u_sb[:], in_=kp_sb[:, X_END:KL], func=Tanh)
        return u_sb

    def emit_qloop(b, hg, kp_sb, u_sb, next_kp, prev_state):
        """features + scores for one (batch, head-group); emits the previous
        chunk's exp and the NEXT chunk's u mid-loop (keeps ACT streaming)."""
        next_u = [None]
        # scores PSUM: [128=kpos, (kc, q, h)]
        sc_ps = sc_pool.tile([128, KC * Q * 4], F32)
        sc_r = sc_ps[:].rearrange("p (kc q h) -> p kc q h", kc=KC, q=Q, h=4)

        exp_done = [None]
        for q in range(Q):
            if q == 3 and prev_state is not None:
                exp_done[0] = emit_exp(prev_state)
            if q == 5 and next_kp is not None:
                next_u[0] = emit_u(next_kp)
            col = b * Q + q
            feat = feat_pool.tile([128, KL], BF16)
            # ACT direct share
            nc.scalar.activation(
                out=feat[:, 0:X_END],
                in_=kp_sb[:, 0:X_END],
                func=Tanh,
                bias=qpT[hg][:, col : col + 1],
            )
            # Mobius share: feat = (u + a) / (1 + a*u)
            acol = a_t[hg][:, col : col + 1]
            den = mob_pool.tile([128, W], BF16, tag="den", name="den")
            num = mob_pool.tile([128, W], BF16, tag="num", name="num")
            r = mob_pool.tile([128, W], BF16, tag="r", name="r")
            nc.vector.tensor_scalar(
                out=den[:], in0=u_sb[:], scalar1=acol,
                scalar2=1.0, op0=MULT, op1=ADD,
            )
            nc.vector.tensor_scalar(
                out=num[:], in0=u_sb[:], scalar1=acol,
                scalar2=0.0, op0=ADD, op1=ADD,
            )
            with nc.allow_low_precision(reason="bf16 attn scores, tol 2e-2"):
                nc.vector.reciprocal(out=r[:], in_=den[:])
            wp = P_SPLIT - X_END
            nc.vector.tensor_tensor(
                out=feat[:, X_END:P_SPLIT], in0=num[:, 0:wp], in1=r[:, 0:wp],
                op=MULT,
            )
            if wp < W:
                nc.gpsimd.tensor_tensor(
                    out=feat[:, P_SPLIT:KL], in0=num[:, wp:W], in1=r[:, wp:W],
                    op=MULT,
                )
            # score matmuls (feat stationary, S moving)
            for kc in range(KC):
                nc.tensor.matmul(
                    out=sc_r[:, kc, q, :],
                    lhsT=feat[:, kc * 128 : (kc + 1) * 128],
                    rhs=S[:],
                    start=True,
                    stop=True,
                )

        return sc_ps, exp_done[0], next_u[0]

    def emit_soft(b, hg, exp_sb):
        Zt = soft_pool.tile([128, KC * 4], F32, tag="Zt", name="Zt")
        exp_khq = exp_sb[:].rearrange("p (kc q h) -> p kc h q", kc=KC, q=Q, h=4)
        nc.vector.tensor_reduce(
            out=Zt[:], in_=exp_khq, axis=mybir.AxisListType.X, op=mybir.AluOpType.add
        )
        invZ = soft_pool.tile([128, KC * 4], BF16, tag="invZ", name="invZ")
        with nc.allow_low_precision(reason="bf16 softmax, tol 2e-2"):
            nc.vector.reciprocal(out=invZ[:], in_=Zt[:])
        if hg == 0:
            en_tiles[b] = en_pool.tile([128, KC * 2 * Q * 4], BF16, name="en_b")
        en_b = en_tiles[b]
        en_view = en_b[:].rearrange(
            "p (kc g q h) -> p g kc q h", kc=KC, g=2, q=Q, h=4
        )
        en_dst = en_view[:, hg]
        in0 = exp_sb[:].rearrange("p (kc q h) -> p kc q h", kc=KC, q=Q, h=4)
        iz = invZ[:].rearrange("p (kc h) -> p kc h", kc=KC, h=4)
        in1 = bass.AP(
            tensor=iz.tensor,
            offset=iz.offset,
            ap=[list(iz.ap[0]), list(iz.ap[1]), [0, Q], list(iz.ap[2])],
        )
        nc.vector.tensor_tensor(out=en_dst, in0=in0, in1=in1, op=MULT)

    def emit_tmp(b):
        """tmpT[c, (g,q,h)] = values[b].T @ en  (contract over k)."""
        en_b = en_tiles[b]
        en_kc = en_b[:].rearrange("p (kc ghq) -> p kc ghq", kc=KC, ghq=2 * Q * 4)
        tmpT_ps = small_tile[:, 64:192]
        for ch in range(2):
            for kc in range(KC):
                nc.tensor.matmul(
                    out=tmpT_ps[:, ch * 64 : (ch + 1) * 64],
                    lhsT=v_all[:, (b * KC + kc) * D + ch * 128 :
                               (b * KC + kc) * D + (ch + 1) * 128],
                    rhs=en_kc[:, kc],
                    start=(kc == 0),
                    stop=(kc == KC - 1),
                )
        tmpT_sb = consts.tile([128, 128], BF16, tag=f"tmpT{b}", name=f"tmpT{b}")
        nc.vector.tensor_copy(out=tmpT_sb[:], in_=tmpT_ps[:])
        return tmpT_sb

    # WvT needed inside the loop for the ao accumulation
    WvT = wtrans("WvT", Wv)   # WvT[ch][c_lo, j] = Wv[j, ch*128+c_lo]

    # chained load order: K0t -> K1 -> V0 -> K2 -> V1 -> K3 -> V2 -> V3
    link = emit_keys_tr(0)
    for b, is_keys in ((1, True), (0, False), (2, True), (1, False),
                       (3, True), (2, False), (3, False)):
        if is_keys:
            emit_keys_cast(b, dep=link)
            link = emit_keys_tr(b)
        else:
            link = emit_values(b, dep=link)
    last_load = link

    # ao[(hh,dh), (hg, b, q)] + tmpT scratch: one persistent PSUM bank
    small_tile = small_ps.tile([128, 192], F32, tag="small", name="small_tile")
    ao_ps = small_tile[:, 0 : 2 * NB * Q]
    ao_state = {"prev": None}

    def emit_ao(b, tmpT_sb):
        t_ghq = tmpT_sb[:].rearrange(
            "p (ch g q h) -> p ch g h q", ch=2, g=2, q=Q, h=4
        )
        for hg in range(2):
            for hh in range(4):
                j0 = (hg * 4 + hh) * DH
                c0 = hg * NB * Q + b * Q
                for ch in range(2):
                    mm = nc.tensor.matmul(
                        out=ao_ps[hh * 32 : (hh + 1) * 32, c0 : c0 + Q],
                        lhsT=WvT[ch][:, j0 : j0 + DH],
                        rhs=t_ghq[:, ch, hg, hh],
                        start=(ch == 0),
                        stop=(ch == 1),
                        tile_position=(0, hh * 32),
                        skip_group_check=True,
                    )
                    if ao_state["prev"] is not None:
                        tile.add_dep_helper(
                            mm.ins, ao_state["prev"], sync=False,
                            reason="ao group order",
                        )
                    ao_state["prev"] = mm.ins

    tmpT_sbs = [None] * NB
    chunks = [(b, hg) for b in range(NB) for hg in range(2)]
    kps = [emit_kp(*chunks[0]), emit_kp(*chunks[1])]  # 2-ahead staging
    u_sb = emit_u(kps[0])
    prev_sc = None   # previous chunk's score psum (exp not yet emitted)
    prev_id = None   # its (b, hg)
    for i, (b, hg) in enumerate(chunks):
        if i + 2 < len(chunks):
            kps.append(emit_kp(*chunks[i + 2]))
        next_kp = kps[i + 1] if i + 1 < len(chunks) else None
        sc_ps, prev_exp, next_u = emit_qloop(b, hg, kps[i], u_sb, next_kp, prev_sc)
        u_sb = next_u
        if prev_exp is not None:
            emit_soft(*prev_id, prev_exp)
            pb, phg = prev_id
            if phg == 1:
                tmpT_sbs[pb] = emit_tmp(pb)
                emit_ao(pb, tmpT_sbs[pb])
        prev_sc, prev_id = sc_ps, (b, hg)
    last_exp = emit_exp(prev_sc)
    emit_soft(*prev_id, last_exp)
    tmpT_sbs[NB - 1] = emit_tmp(NB - 1)
    emit_ao(NB - 1, tmpT_sbs[NB - 1])

    # -------------------------------------------------- tail weights
    WoT = wtrans("WoT", Wo, dep=last_load)  # WoT[ch][j_lo, jo] = Wo[jo, ch*128+j_lo]
    fcw_nat = []
    for r in range(2):
        t = consts.tile([128, Q * D], F32, tag=f"fcwnat{r}", name=f"fcwnat{r}")
        dma = nc.sync.dma_start(out=t[:], in_=fcW[r * 128 : (r + 1) * 128, :])
        tile.add_dep_helper(dma.ins, last_load.ins, reason="dma order")
        fcw_nat.append(t)
    fcwT = [
        consts.tile([128, D], BF16, tag=f"fcwT{t}", name=f"fcwT{t}")
        for t in range(16)
    ]
    for t in range(16):
        ps = psA.tile([128, D], F32, tag="psA", name=f"fcwTps{t}")
        for r in range(2):
            nc.tensor.transpose(
                out=ps[:, r * 128 : (r + 1) * 128],
                in_=fcw_nat[r][:, t * 128 : (t + 1) * 128],
                identity=id128f[:],
            )
        if t % 2 == 0:
            nc.scalar.copy(out=fcwT[t][:], in_=ps[:])
        else:
            nc.vector.tensor_copy(out=fcwT[t][:], in_=ps[:])

    # ------------------------------------------------------------------ tail
    aoT = [
        consts.tile([128, NB * Q], BF16, tag=f"aoT{hg}", name=f"aoT{hg}")
        for hg in range(2)
    ]
    for hg in range(2):
        nc.vector.tensor_copy(
            out=aoT[hg][:], in_=ao_ps[:, hg * NB * Q : (hg + 1) * NB * Q]
        )

    # out2T[m][jo_lo, (b,q)] = (ao @ Wo.T) transposed
    o2T = [
        consts.tile([128, NB * Q], BF16, tag=f"o2T{m}", name=f"o2T{m}")
        for m in range(2)
    ]
    for m in range(2):
        o2_ps = psA.tile([128, NB * Q], F32, tag="psA", name="o2_ps")
        for ch in range(2):
            nc.tensor.matmul(
                out=o2_ps[:],
                lhsT=WoT[ch][:, m * 128 : (m + 1) * 128],
                rhs=aoT[ch][:],
                start=(ch == 0),
                stop=(ch == 1),
            )
        nc.vector.tensor_copy(out=o2T[m][:], in_=o2_ps[:])

    # fc: y[b, f] = sum_{q,jo} out2[b,q,jo] * fcW[f, q*256+jo]
    y_ps = psA.tile([NB, D], F32, tag="psA", name="y_ps")
    for t in range(16):
        qq, m = t // 2, t % 2
        lhsT = o2T[m][:].rearrange("p (b q) -> p q b", b=NB, q=Q)[:, qq, :]
        nc.tensor.matmul(
            out=y_ps[:], lhsT=lhsT, rhs=fcwT[t][:], start=(t == 0), stop=(t == 15)
        )
    y_sb = consts.tile([NB, D], F32, tag="y_sb", name="y_sb")
    nc.vector.tensor_tensor(out=y_sb[:], in0=y_ps[:], in1=fcb_sb[:], op=ADD)
    nc.sync.dma_start(out=out, in_=y_sb[:])

    for p in pools:
        p.release()


_NC_CACHE = None


def _get_nc():
    global _NC_CACHE
    if _NC_CACHE is None:
        nc = bacc.Bacc(
            "TRN2", target_bir_lowering=False, debug=False, num_devices=NCORES
        )
        with tile.TileContext(nc) as tc:
            _emit(tc)
        nc.compile()
        _NC_CACHE = nc
    return _NC_CACHE


def _in_maps(inputs):
    f32 = lambda x: np.ascontiguousarray(np.asarray(x), dtype=np.float32)
    queries = f32(inputs["queries"])
    keys = f32(inputs["keys"])
    values = f32(inputs["values"])
    shared = {
        "Wq": f32(inputs["Wq"]),
        "Wk": f32(inputs["Wk"]),
        "Wv": f32(inputs["Wv"]),
        "Wo": f32(inputs["Wo"]),
        "wv_score": f32(inputs["wv_score"]),
        "fcW": f32(inputs["fcW"]),
        "fcb": f32(inputs["fcb"]),
    }
    maps = []
    for c in range(NCORES):
        sl = slice(c * NB, (c + 1) * NB)
        maps.append(
            {
                "queries": np.ascontiguousarray(queries[sl]),
                "keys": np.ascontiguousarray(keys[sl]),
                "values": np.ascontiguousarray(values[sl]),
                **shared,
            }
        )
    return maps


def run(inputs, trace=False):
    nc = _get_nc()
    res = run_bass_kernel_spmd(
        nc, _in_maps(inputs), core_ids=list(range(NCORES)), trace=trace
    )
    outp = np.concatenate([res.results[c]["out"] for c in range(NCORES)], axis=0)
    return outp, res.exec_time_ns


def run_sim(inputs):
    import concourse.bass_interp as bass_interp

    nc = _get_nc()
    sim = bass_interp.CoreSim(nc)
    for k, v in _in_maps(inputs)[0].items():
        sim.tensor(k)[:] = v
    sim.simulate()
    return np.array(sim.tensor("out"))


def kernel(**inputs):
    return run(inputs, trace=False)[0]
